# revision 1
# baseline (speedup 1.0000x reference)
"""Trainium2 Bass kernel for nn_ChunkedCrossAttention_85907935855128.

Self-contained: hardcodes shapes/sharding. Accepts FULL inputs, returns FULL output.
Shards the fused (b*k_chunks) chunk axis across 8 NeuronCores; weights replicated.

Per-core dataflow (all matmul layouts chosen so no on-device transposition of the
big activations is needed; host passes x/context pre-transposed, dim-major):
  qT/kT inner-major via fp32r matmuls (lhsT=W tile, rhs=xT/ctxT), v token-major
  (lhsT=ctxT tile, rhs=Wv). Rope on k = cos*k + sin*(signed-perm matmul on PE).
  Rope on q is identity except each chunk's token 0 (causal shift zeroes the rest
  of the shifted q_pos_emb). Attention in bf16: simT[j,(h,i)] psum -> ACT exp ->
  o[i,65] psum (col 64 = softmax sum via ones column in v_aug) -> reciprocal *
  per-head -> PE-transpose -> fp32r out-projection + bias.
"""
import os
# bass2jax executes via the axon PJRT platform; a CPU pin would hide the cores.
if os.environ.get("JAX_PLATFORMS", "") in ("cpu",):
    del os.environ["JAX_PLATFORMS"]

import numpy as np

import concourse.bacc as bacc
import concourse.bass as bass
import concourse.mybir as mybir
import concourse.tile as tile
from concourse.bass_utils import run_bass_kernel_spmd
from concourse.masks import make_identity

F32 = mybir.dt.float32
F32R = mybir.dt.float32r
BF16 = mybir.dt.bfloat16

CS, CP, H, DH = 64, 63, 8, 64
SCALE = DH ** -0.5
N_CORES = 8
B, N, DIM = 4, 4096, 1024
K_CHUNKS, R, RLEN = 64, 2, 128
TK = R * RLEN                 # 256 ctx tokens / chunk
BK = B * K_CHUNKS             # 256 chunks
CPC = BK // N_CORES           # 32 chunks / core
TQ = CPC * CS                 # 2048 q tokens / core
TCTX = CPC * TK               # 8192 ctx tokens / core
INNER = H * DH                # 512
QG = 4                        # chunks per q-projection group (N=256)
NQG = CPC // QG               # 8 q groups / core


def _build_bass(cpc=CPC, num_devices=N_CORES, do_rope=True, do_attn=True, do_out=True, attn_stop=3):
    tq = cpc * CS
    tctx = cpc * TK
    nqg = cpc // QG
    nc = bacc.Bacc("TRN2", target_bir_lowering=False, debug=False,
                   num_devices=num_devices)

    xT = nc.dram_tensor("xT", (DIM, tq), F32, kind="ExternalInput")
    ctxT = nc.dram_tensor("ctxT", (DIM, tctx), F32, kind="ExternalInput")
    Wq = nc.dram_tensor("Wq", (DIM, INNER), F32, kind="ExternalInput")   # pre-scaled
    Wk = nc.dram_tensor("Wk", (DIM, INNER), F32, kind="ExternalInput")
    Wv = nc.dram_tensor("Wv", (DIM, INNER), F32, kind="ExternalInput")
    Wo = nc.dram_tensor("Wo", (INNER, DIM), F32, kind="ExternalInput")
    bo = nc.dram_tensor("bo", (DIM,), F32, kind="ExternalInput")
    cos_kT = nc.dram_tensor("cos_kT", (64, 128), F32, kind="ExternalInput")
    sin_kT = nc.dram_tensor("sin_kT", (64, 128), F32, kind="ExternalInput")
    Pm = nc.dram_tensor("Pm", (64, 64), F32, kind="ExternalInput")
    nullkT = nc.dram_tensor("nullkT", (64, 8), F32, kind="ExternalInput")
    nullv_aug = nc.dram_tensor("nullv_aug", (1, 8 * 65), F32, kind="ExternalInput")
    cos_q0 = nc.dram_tensor("cos_q0", (64, 1), F32, kind="ExternalInput")
    sin_q0s = nc.dram_tensor("sin_q0s", (64, 1), F32, kind="ExternalInput")
    out = nc.dram_tensor("out", (tq, DIM), F32, kind="ExternalOutput")

    with tile.TileContext(nc) as tc:
        with tc.tile_pool(name="consts", bufs=1) as cp_, \
             tc.tile_pool(name="wk", bufs=2) as wk, \
             tc.tile_pool(name="psb", bufs=3, space="PSUM") as psb, \
             tc.tile_pool(name="pst", bufs=1, space="PSUM") as pst:

            # ---- constants ----
            wq_sb = cp_.tile([128, 8, INNER], F32R)
            nc.sync.dma_start(out=wq_sb, in_=Wq[:, :].rearrange(
                "(dt p) i -> p dt i", p=128).bitcast(F32R))
            wk_sb = cp_.tile([128, 8, INNER], F32R)
            nc.sync.dma_start(out=wk_sb, in_=Wk[:, :].rearrange(
                "(dt p) i -> p dt i", p=128).bitcast(F32R))
            wv_sb = cp_.tile([128, 8, INNER], F32R)
            nc.sync.dma_start(out=wv_sb, in_=Wv[:, :].rearrange(
                "(dt p) i -> p dt i", p=128).bitcast(F32R))
            wo_sb = cp_.tile([128, 4, DIM], F32R)
            nc.sync.dma_start(out=wo_sb, in_=Wo[:, :].rearrange(
                "(et p) c -> p et c", p=128).bitcast(F32R))

            bo_sb = cp_.tile([128, DIM], F32)
            nc.sync.dma_start(out=bo_sb, in_=bass.AP(
                tensor=bo, offset=0, ap=[[0, 128], [1, DIM]]))

            cosk_sb = cp_.tile([64, 128], F32)
            nc.sync.dma_start(out=cosk_sb, in_=cos_kT[:, :])
            sink_sb = cp_.tile([64, 128], F32)
            nc.sync.dma_start(out=sink_sb, in_=sin_kT[:, :])
            cosq_sb = cp_.tile([64, 1], F32)
            nc.sync.dma_start(out=cosq_sb, in_=cos_q0[:, :])
            sinq_sb = cp_.tile([64, 1], F32)
            nc.sync.dma_start(out=sinq_sb, in_=sin_q0s[:, :])

            pm_f32 = cp_.tile([64, 64], F32)
            nc.sync.dma_start(out=pm_f32, in_=Pm[:, :])
            pm_bf = cp_.tile([64, 64], BF16)
            nc.vector.tensor_copy(pm_bf, pm_f32)

            nullk_f32 = cp_.tile([64, 8], F32)
            nc.sync.dma_start(out=nullk_f32, in_=nullkT[:, :])
            nullk_bf = cp_.tile([64, 8], BF16)
            nc.vector.tensor_copy(nullk_bf, nullk_f32)

            nullv_f32 = cp_.tile([1, 8, 65], F32)
            nc.sync.dma_start(out=nullv_f32, in_=nullv_aug[:, :].rearrange(
                "o (h w) -> o h w", h=8))
            nullv_bf = cp_.tile([1, 8, 65], BF16)
            nc.vector.tensor_copy(nullv_bf, nullv_f32)

            ident = cp_.tile([128, 128], F32)
            make_identity(nc, ident)

            for g in range(nqg):          # 8 groups of 4 chunks
                # ---- q projection for this group: qT [512, 256] ----
                xT_sb = wk.tile([128, 8, QG * CS], F32R, tag="xT", bufs=1)
                nc.sync.dma_start(out=xT_sb, in_=xT[:, :].rearrange(
                    "(dt p) t -> p dt t", p=128)[:, :, g * QG * CS:(g + 1) * QG * CS]
                    .bitcast(F32R))
                qps = psb.tile([128, 4, QG * CS], F32, tag="ps", name=f"qps{g}")
                for it in range(4):
                    for dt in range(8):
                        nc.tensor.matmul(
                            qps[:, it, :],
                            wq_sb[:, dt, it * 128:(it + 1) * 128],
                            xT_sb[:, dt, :],
                            start=(dt == 0), stop=(dt == 7))
                qT_sb = wk.tile([64, 8, QG * CS], BF16, tag="qT", bufs=2)
                for it in range(4):
                    nc.vector.tensor_copy(qT_sb[:, 2 * it, :], qps[0:64, it, :])
                    nc.vector.tensor_copy(qT_sb[:, 2 * it + 1, :], qps[64:128, it, :])
                # rope-q: fix token 0 of each chunk (cols ::CS)
                qcols = qT_sb[:, :, :].rearrange(
                    "p h (c w) -> p h c w", w=CS)[:, :, :, 0]   # [64, 8, QG]
                t1q = wk.tile([64, 8, QG], BF16, tag="t1q", bufs=2)
                nc.vector.tensor_mul(
                    t1q, qcols,
                    cosq_sb.unsqueeze(2).broadcast_to((64, 8, QG)))
                t2q = wk.tile([64, 8, QG], BF16, tag="t2q", bufs=2)
                for (dst, src) in ((0, 32), (32, 0)):
                    nc.vector.tensor_mul(
                        t2q[dst:dst + 32, :, :],
                        qT_sb[:, :, :].rearrange(
                            "p h (c w) -> p h c w", w=CS)[src:src + 32, :, :, 0],
                        sinq_sb[src:src + 32, :].unsqueeze(2)
                        .broadcast_to((32, 8, QG)))
                nc.vector.tensor_add(qcols, t1q, t2q)

                # ---- null sims for group: expn_g [1, 8, 256] bf16 ----
                expn_g = wk.tile([1, 8, QG * CS], BF16, tag="expn", bufs=2)
                for h in range(H):
                    nps = pst.tile([1, QG * CS], F32, tag="pst", name=f"nps{g}_{h}")
                    nc.tensor.matmul(
                        nps[:, :],
                        nullk_bf[:, h:h + 1],
                        qT_sb[:, h, :],
                        start=True, stop=True)
                    nc.scalar.activation(expn_g[:, h, :], nps[:, :],
                                         mybir.ActivationFunctionType.Exp)

                for pp in range(QG // 2):
                    cpair = g * QG + pp * 2   # first chunk of the pair
                    # ---- load ctxT pair slice [1024, 512] ----
                    ctx_sb = wk.tile([128, 8, 2 * TK], F32R, tag="ctx", bufs=2)
                    nc.sync.dma_start(out=ctx_sb, in_=ctxT[:, :].rearrange(
                        "(dt p) t -> p dt t", p=128)
                        [:, :, cpair * TK:(cpair + 2) * TK].bitcast(F32R))

                    # ---- k projection for the pair (N=512) ----
                    kps_a = psb.tile([128, 2, 2 * TK], F32, tag="ps", name=f"kpsa{cpair}")
                    kps_b = psb.tile([128, 2, 2 * TK], F32, tag="ps", name=f"kpsb{cpair}")
                    for it in range(4):
                        kp_t = (kps_a, kps_b)[it // 2]
                        for dt in range(8):
                            nc.tensor.matmul(
                                kp_t[:, it % 2, :],
                                wk_sb[:, dt, it * 128:(it + 1) * 128],
                                ctx_sb[:, dt, :],
                                start=(dt == 0), stop=(dt == 7))
                    kraw = wk.tile([64, 8, 2 * TK], BF16, tag="kraw", bufs=2)
                    for it in range(4):
                        kp_t = (kps_a, kps_b)[it // 2]
                        nc.scalar.copy(kraw[:, 2 * it, :], kp_t[0:64, it % 2, :])
                        nc.scalar.copy(kraw[:, 2 * it + 1, :], kp_t[64:128, it % 2, :])

                    # ---- rope-k: perm matmul + combine (pair) ----
                    kpps_t = [
                        psb.tile([64, 2, 2 * TK], F32, tag="ps",
                                 name=f"kpps{q}_{cpair}")
                        for q in range(4)]
                    for q4 in range(8):
                        dst_t = kpps_t[q4 // 2]
                        nc.tensor.matmul(
                            dst_t[:, :, :].rearrange("p h t -> p (h t)")
                            [:, (q4 % 2) * 512:(q4 % 2 + 1) * 512],
                            pm_bf,
                            kraw[:, :, :].rearrange("p h t -> p (h t)")
                            [:, q4 * 512:(q4 + 1) * 512],
                            start=True, stop=True)
                    t1k = wk.tile([64, 8, 2 * TK], BF16, tag="t1k", bufs=1)
                    nc.vector.tensor_mul(
                        t1k[:, :, :].rearrange("p h (rep c) -> p h rep c", rep=4),
                        kraw[:, :, :].rearrange("p h (rep c) -> p h rep c", rep=4),
                        cosk_sb.unsqueeze(1).unsqueeze(2)
                        .broadcast_to((64, 8, 4, 128)))
                    t2k = wk.tile([64, 8, 2 * TK], BF16, tag="t2k", bufs=1)
                    for q, kp_t in enumerate(kpps_t):
                        nc.vector.tensor_mul(
                            t2k[:, q * 2:(q + 1) * 2, :].rearrange(
                                "p h (rep c) -> p h rep c", rep=4),
                            kp_t[:, :, :].rearrange(
                                "p h (rep c) -> p h rep c", rep=4),
                            sink_sb.unsqueeze(1).unsqueeze(2)
                            .broadcast_to((64, 2, 4, 128)))
                    kT_bf = wk.tile([64, 8, 2 * TK], BF16, tag="kT", bufs=2)
                    nc.vector.tensor_add(kT_bf, t1k, t2k)

                    # two chunks of attention per pair
                    for sub in range(2):
                        cc = pp * 2 + sub
                        c = g * QG + cc
                        # ---- v projection -> v_aug bf16 [128, 2, 8, 65] ----
                        vps = psb.tile([128, 2, INNER], F32, tag="ps", name=f"vps{c}")
                        for tg in range(2):
                            for dt in range(8):
                                nc.tensor.matmul(
                                    vps[:, tg, :],
                                    ctx_sb[:, dt, sub * TK + tg * 128:
                                           sub * TK + (tg + 1) * 128],
                                    wv_sb[:, dt, :],
                                    start=(dt == 0), stop=(dt == 7))
                        v_aug = wk.tile([128, 2, 8, 65], BF16, tag="v_aug", bufs=2)
                        nc.scalar.copy(
                            v_aug[:, :, :, 0:64],
                            vps[:, :, :].rearrange("p tg (h w) -> p tg h w", h=8))
                        nc.gpsimd.memset(v_aug[:, :, :, 64:65], 1.0)

                        if not do_attn:
                            continue
                        # ---- sim matmuls: simT [128j, 2jg, (h,i)] ----
                        sps = psb.tile([128, 2, 512], F32, tag="ps", name=f"sps{c}")
                        for h in range(H):
                            for jg in range(2):
                                nc.tensor.matmul(
                                    sps[:, jg, h * 64:(h + 1) * 64],
                                    kT_bf[:, h, sub * TK + jg * 128:
                                          sub * TK + (jg + 1) * 128],
                                    qT_sb[:, h, cc * CS:(cc + 1) * CS],
                                    start=True, stop=True)
                        if attn_stop == 0:
                            dbg = wk.tile([64, DIM], F32, tag="out_sb", bufs=2)
                            nc.vector.tensor_copy(dbg[:, 0:512], sps[0:64, 0, :])
                            nc.vector.memset(dbg[:, 512:], 0.0)
                            nc.sync.dma_start(out=out[c * CS:(c + 1) * CS, :], in_=dbg)
                            continue
                        expT = wk.tile([128, 2, 512], BF16, tag="expT", bufs=2)
                        nc.scalar.activation(expT, sps,
                                             mybir.ActivationFunctionType.Exp)
                        if attn_stop == 1:
                            dbg = wk.tile([64, DIM], F32, tag="out_sb", bufs=2)
                            nc.vector.tensor_copy(dbg[:, 0:512], expT[0:64, 0, :])
                            nc.vector.memset(dbg[:, 512:], 0.0)
                            nc.sync.dma_start(out=out[c * CS:(c + 1) * CS, :], in_=dbg)
                            continue

                        # ---- o matmuls [64i, 65] per head (col 64 = softmax sum) ----
                        ops_ = psb.tile([64, 8, 128], F32, tag="ps", name=f"ops{c}")
                        for h in range(H):
                            dst = ops_[:, h, 0:65]
                            for jg in range(2):
                                nc.tensor.matmul(
                                    dst,
                                    expT[:, jg, h * 64:(h + 1) * 64],
                                    v_aug[:, jg, h, :],
                                    start=(jg == 0), stop=False)
                            nc.tensor.matmul(
                                dst,
                                expn_g[0:1, h, c * CS - g * QG * CS:
                                       c * CS - g * QG * CS + CS],
                                nullv_bf[0:1, h, :],
                                start=False, stop=True)

                        if attn_stop == 2:
                            dbg = wk.tile([64, DIM], F32, tag="out_sb", bufs=2)
                            nc.vector.tensor_copy(dbg[:, 0:128], ops_[:, 0, :])
                            nc.vector.memset(dbg[:, 128:], 0.0)
                            nc.sync.dma_start(out=out[c * CS:(c + 1) * CS, :], in_=dbg)
                            continue
                        # ---- normalize (batched) into pair buffer ----
                        rcol = wk.tile([64, 8], F32, tag="rcol", bufs=2)
                        nc.vector.reciprocal(rcol, ops_[:, :, 64])
                        if sub == 0:
                            o_pair = wk.tile([128, 8, 64], F32, tag="o_pair",
                                             bufs=2)
                        nc.vector.tensor_mul(
                            o_pair[sub * 64:(sub + 1) * 64, :, :],
                            ops_[:, :, 0:64],
                            rcol.unsqueeze(2).broadcast_to((64, 8, 64)))

                        if not do_out:
                            continue
                        if sub == 0:
                            continue
                        # ---- transpose o pair -> oT fp32r [128e, 4et, 128t] ----
                        otr = pst.tile([128, 4, 128], F32, tag="pst",
                                       name=f"otr{cpair}")
                        for et in range(4):
                            nc.tensor.transpose(
                                otr[:, et, :],
                                o_pair[:, 2 * et:2 * et + 2, :],
                                ident)
                        oT_sb = wk.tile([128, 4, 128], F32R, tag="oT", bufs=2)
                        nc.vector.tensor_copy(oT_sb, otr)

                        # ---- out projection + bias (pair, M=128) ----
                        outps = psb.tile([128, DIM], F32, tag="ps",
                                         name=f"outps{cpair}")
                        for co in range(2):
                            for et in range(4):
                                nc.tensor.matmul(
                                    outps[:, co * 512:(co + 1) * 512],
                                    oT_sb[:, et, :],
                                    wo_sb[:, et, co * 512:(co + 1) * 512],
                                    start=(et == 0), stop=(et == 3))
                        out_sb = wk.tile([128, DIM], F32, tag="out_sb", bufs=2)
                        nc.vector.tensor_add(out_sb, outps, bo_sb)
                        nc.sync.dma_start(
                            out=out[cpair * CS:(cpair + 2) * CS, :], in_=out_sb)

    nc.compile()
    return nc


_CACHED_NC = None


def _get_nc():
    global _CACHED_NC
    if _CACHED_NC is None:
        _CACHED_NC = _build_bass()
    return _CACHED_NC


def kernel(x, context, q_pos_emb, k_pos_emb, Wq, Wk, Wv, Wo, bo, null_k, null_v):
    x = np.asarray(x, dtype=np.float32)
    context = np.asarray(context, dtype=np.float32)
    q_pos_emb = np.asarray(q_pos_emb, dtype=np.float32)
    k_pos_emb = np.asarray(k_pos_emb, dtype=np.float32)
    Wq = np.asarray(Wq, dtype=np.float32)
    Wk = np.asarray(Wk, dtype=np.float32)
    Wv = np.asarray(Wv, dtype=np.float32)
    Wo = np.asarray(Wo, dtype=np.float32)
    bo = np.asarray(bo, dtype=np.float32)
    null_k = np.asarray(null_k, dtype=np.float32)
    null_v = np.asarray(null_v, dtype=np.float32)

    # ---- host marshalling (layout only + tiny rope tables) ----
    xs = np.zeros_like(x)
    xs[:, : N - CP] = x[:, CP:]
    xc = xs.reshape(BK, CS, DIM)
    ctx = context.reshape(BK, TK, DIM)

    Wq_s = np.ascontiguousarray(Wq * SCALE)

    qpe63 = q_pos_emb[0, 0, CP]
    cos_q0 = np.cos(qpe63)[:, None].astype(np.float32)          # [64, 1]
    sgn = np.where(np.arange(64) < 32, -1.0, 1.0)
    sin_q0s = (np.sin(qpe63) * sgn)[:, None].astype(np.float32)
    # permuted so the partition-shifted mul reads table at the src base
    # partition (BIR requires equal base partitions for two SBUF inputs)
    sp = np.empty_like(sin_q0s)
    sp[0:32] = sin_q0s[32:64]; sp[32:64] = sin_q0s[0:32]
    sin_q0s = sp

    kpe = k_pos_emb[0, 0]
    cos_kT = np.ascontiguousarray(np.cos(kpe.T).astype(np.float32))   # [64, 128]
    sin_kT = np.ascontiguousarray(np.sin(kpe.T).astype(np.float32))

    Pm = np.zeros((64, 64), np.float32)
    for rout in range(64):
        if rout < 32:
            Pm[rout + 32, rout] = -1.0
        else:
            Pm[rout - 32, rout] = 1.0

    nullkT = np.ascontiguousarray(null_k.reshape(8, 64).T.astype(np.float32))  # [64, 8]
    nullv_aug = np.zeros((1, 8, 65), np.float32)
    nullv_aug[0, :, :64] = null_v.reshape(8, 64)
    nullv_aug[0, :, 64] = 1.0
    nullv_aug = nullv_aug.reshape(1, 8 * 65)

    shared = {
        "Wq": Wq_s, "Wk": Wk, "Wv": Wv, "Wo": Wo, "bo": bo,
        "cos_kT": cos_kT, "sin_kT": sin_kT, "Pm": Pm,
        "nullkT": nullkT, "nullv_aug": nullv_aug,
        "cos_q0": cos_q0, "sin_q0s": sin_q0s,
    }
    in_maps = []
    for c in range(N_CORES):
        sl = slice(c * CPC, (c + 1) * CPC)
        xT_c = np.ascontiguousarray(xc[sl].reshape(TQ, DIM).T)
        ctxT_c = np.ascontiguousarray(ctx[sl].reshape(TCTX, DIM).T)
        in_maps.append({"xT": xT_c, "ctxT": ctxT_c, **shared})

    nc = _get_nc()
    res = run_bass_kernel_spmd(nc, in_maps, core_ids=list(range(N_CORES)))

    out_full = np.concatenate([res.results[c]["out"] for c in range(N_CORES)],
                              axis=0)                      # [BK*CS, DIM]
    o = out_full.reshape(B, K_CHUNKS * CS, DIM)
    final = np.concatenate(
        [np.zeros((B, CP, DIM), np.float32), o[:, : K_CHUNKS * CS - CP]], axis=1)
    return final



# revision 3
# speedup vs baseline: 1.1286x; 1.1286x over previous
"""Trainium2 Bass kernel for nn_ChunkedCrossAttention_85907935855128.

Self-contained: hardcodes shapes/sharding. Accepts FULL inputs, returns FULL output.
Shards the fused (b*k_chunks) chunk axis across 8 NeuronCores; weights replicated.

Per-core dataflow:
  q/k/v projections run as fp8-E4M3 DoubleRow matmuls with a 3-term hi/lo
  error-compensation split (act_hi@W_hi + act_lo@W_hi + act_hi@W_lo), which
  the host prepares: activations split into e4m3 hi + residual lo, weights
  pre-scaled (x32 for Wk/Wv, x256*SCALE for Wq) so they quantize outside the
  e4m3 denormal range. Scale compensation is folded into constants: 1/(32*256)
  into the k rope tables, 1/256 into null_k, 1/32 into the v psum->sbuf copy.
  Rope on k = cos*k + sin*shift(k) where the rotate-half shift is done by an
  SBUF->SBUF DMA partition swap (keeps it off the PE/DVE critical path).
  Attention (sim/exp/o) in bf16 with softmax sums via a ones column in v_aug;
  out-projection in f32r; bias add + bf16 cast on gpsimd; bf16 output.
"""
import os
# bass2jax executes via the axon PJRT platform; a CPU pin would hide the cores.
if os.environ.get("JAX_PLATFORMS", "") in ("cpu",):
    del os.environ["JAX_PLATFORMS"]

import numpy as np
import ml_dtypes

import concourse.bacc as bacc
import concourse.bass as bass
import concourse.mybir as mybir
import concourse.tile as tile
from concourse.bass_utils import run_bass_kernel_spmd
from concourse.masks import make_identity

F32 = mybir.dt.float32
F32R = mybir.dt.float32r
BF16 = mybir.dt.bfloat16
F8 = mybir.dt.float8e4
DR = mybir.MatmulPerfMode.DoubleRow

CS, CP, H, DH = 64, 63, 8, 64
SCALE = DH ** -0.5
N_CORES = 8
B, N, DIM = 4, 4096, 1024
K_CHUNKS, R, RLEN = 64, 2, 128
TK = R * RLEN                 # 256 ctx tokens / chunk
BK = B * K_CHUNKS             # 256 chunks
CPC = BK // N_CORES           # 32 chunks / core
TQ = CPC * CS                 # 2048 q tokens / core
TCTX = CPC * TK               # 8192 ctx tokens / core
INNER = H * DH                # 512
QG = 8                        # chunks per q-projection group (512 tokens)
NQG = CPC // QG               # 4 q groups / core
WS_KV = 32.0                  # host pre-scale on Wk/Wv before e4m3
WS_Q = 256.0                  # host pre-scale on Wq*SCALE before e4m3


def _build_bass(num_devices=N_CORES):
    nc = bacc.Bacc("TRN2", target_bir_lowering=False, debug=False,
                   num_devices=num_devices)

    xT_hi = nc.dram_tensor("xT_hi", (DIM, TQ), F8, kind="ExternalInput")
    xT_lo = nc.dram_tensor("xT_lo", (DIM, TQ), F8, kind="ExternalInput")
    ctxT_hi = nc.dram_tensor("ctxT_hi", (DIM, TCTX), F8, kind="ExternalInput")
    ctxT_lo = nc.dram_tensor("ctxT_lo", (DIM, TCTX), F8, kind="ExternalInput")
    wq_hi = nc.dram_tensor("wq_hi", (DIM, INNER), F8, kind="ExternalInput")
    wq_lo = nc.dram_tensor("wq_lo", (DIM, INNER), F8, kind="ExternalInput")
    wk_hi = nc.dram_tensor("wk_hi", (DIM, INNER), F8, kind="ExternalInput")
    wk_lo = nc.dram_tensor("wk_lo", (DIM, INNER), F8, kind="ExternalInput")
    wv_hi = nc.dram_tensor("wv_hi", (DIM, INNER), F8, kind="ExternalInput")
    wv_lo = nc.dram_tensor("wv_lo", (DIM, INNER), F8, kind="ExternalInput")
    Wo = nc.dram_tensor("Wo", (INNER, DIM), F32, kind="ExternalInput")
    bo = nc.dram_tensor("bo", (DIM,), F32, kind="ExternalInput")
    cos_kT = nc.dram_tensor("cos_kT", (64, 128), F32, kind="ExternalInput")
    sin_kT = nc.dram_tensor("sin_kT", (64, 128), F32, kind="ExternalInput")
    nullkT = nc.dram_tensor("nullkT", (64, 8), F32, kind="ExternalInput")
    nullv_aug = nc.dram_tensor("nullv_aug", (1, 8 * 65), F32, kind="ExternalInput")
    cos_q0 = nc.dram_tensor("cos_q0", (64, 1), F32, kind="ExternalInput")
    sin_q0s = nc.dram_tensor("sin_q0s", (64, 1), F32, kind="ExternalInput")
    out = nc.dram_tensor("out", (TQ, DIM), BF16, kind="ExternalOutput")

    def w_re(t):
        return t[:, :].rearrange("(dt2 kt p) i -> p dt2 kt i", p=128, kt=2)

    def act_re(t):
        return t[:, :].rearrange("(dt2 kt p) t -> p dt2 kt t", p=128, kt=2)

    with tile.TileContext(nc) as tc:
        with tc.tile_pool(name="consts", bufs=1) as cp_, \
             tc.tile_pool(name="wk", bufs=2) as wk, \
             tc.tile_pool(name="psP", bufs=3, space="PSUM") as psP, \
             tc.tile_pool(name="pst", bufs=2, space="PSUM") as pst:

            # ---- constants ----
            wq_hi_sb = cp_.tile([128, 4, 2, INNER], F8)
            nc.sync.dma_start(out=wq_hi_sb, in_=w_re(wq_hi))
            wq_lo_sb = cp_.tile([128, 4, 2, INNER], F8)
            nc.sync.dma_start(out=wq_lo_sb, in_=w_re(wq_lo))
            wk_hi_sb = cp_.tile([128, 4, 2, INNER], F8)
            nc.sync.dma_start(out=wk_hi_sb, in_=w_re(wk_hi))
            wk_lo_sb = cp_.tile([128, 4, 2, INNER], F8)
            nc.sync.dma_start(out=wk_lo_sb, in_=w_re(wk_lo))
            wv_hi_sb = cp_.tile([128, 4, 2, INNER], F8)
            nc.sync.dma_start(out=wv_hi_sb, in_=w_re(wv_hi))
            wv_lo_sb = cp_.tile([128, 4, 2, INNER], F8)
            nc.sync.dma_start(out=wv_lo_sb, in_=w_re(wv_lo))
            wo_sb = cp_.tile([128, 4, DIM], F32R)
            nc.sync.dma_start(out=wo_sb, in_=Wo[:, :].rearrange(
                "(et p) c -> p et c", p=128).bitcast(F32R))

            bo_sb = cp_.tile([128, DIM], F32)
            nc.sync.dma_start(out=bo_sb, in_=bass.AP(
                tensor=bo, offset=0, ap=[[0, 128], [1, DIM]]))

            cosk_f = cp_.tile([64, 128], F32)
            nc.sync.dma_start(out=cosk_f, in_=cos_kT[:, :])
            cosk_sb = cp_.tile([64, 128], BF16)
            nc.vector.tensor_copy(cosk_sb, cosk_f)
            sink_f = cp_.tile([64, 128], F32)
            nc.sync.dma_start(out=sink_f, in_=sin_kT[:, :])
            sink_sb = cp_.tile([64, 128], BF16)
            nc.vector.tensor_copy(sink_sb, sink_f)

            cosq_sb = cp_.tile([64, 1], F32)
            nc.sync.dma_start(out=cosq_sb, in_=cos_q0[:, :])
            sinq_sb = cp_.tile([64, 1], F32)
            nc.sync.dma_start(out=sinq_sb, in_=sin_q0s[:, :])

            nullk_f32 = cp_.tile([64, 8], F32)
            nc.sync.dma_start(out=nullk_f32, in_=nullkT[:, :])
            nullk_bf = cp_.tile([64, 8], BF16)
            nc.vector.tensor_copy(nullk_bf, nullk_f32)

            nullv_f32 = cp_.tile([1, 8, 65], F32)
            nc.sync.dma_start(out=nullv_f32, in_=nullv_aug[:, :].rearrange(
                "o (h w) -> o h w", h=8))
            nullv_bf = cp_.tile([1, 8, 65], BF16)
            nc.vector.tensor_copy(nullv_bf, nullv_f32)

            ident = cp_.tile([128, 128], BF16)
            make_identity(nc, ident)

            for g in range(NQG):          # 4 groups of 8 chunks
                # ---- q projection (fp8 DR 3-term): qT = 256*SCALE*q ----
                x_hi_sb = wk.tile([128, 4, 2, QG * CS], F8, tag="xhi", bufs=2)
                nc.sync.dma_start(out=x_hi_sb, in_=act_re(xT_hi)
                                  [:, :, :, g * QG * CS:(g + 1) * QG * CS])
                x_lo_sb = wk.tile([128, 4, 2, QG * CS], F8, tag="xlo", bufs=2)
                nc.sync.dma_start(out=x_lo_sb, in_=act_re(xT_lo)
                                  [:, :, :, g * QG * CS:(g + 1) * QG * CS])
                qT_sb = wk.tile([64, 8, QG * CS], BF16, tag="qT", bufs=2)
                for ith in range(2):
                    qps = psP.tile([128, 2, QG * CS], F32, tag="p2",
                                   name=f"qps{g}_{ith}")
                    for it2 in range(2):
                        it = ith * 2 + it2
                        mi = 0
                        for (act_t, w_t) in ((x_hi_sb, wq_hi_sb),
                                             (x_lo_sb, wq_hi_sb),
                                             (x_hi_sb, wq_lo_sb)):
                            for dt2 in range(4):
                                nc.tensor.matmul(
                                    qps[:, it2, :],
                                    w_t[:, dt2, :, it * 128:(it + 1) * 128],
                                    act_t[:, dt2, :, :],
                                    start=(mi == 0), stop=(mi == 11),
                                    perf_mode=DR)
                                mi += 1
                        nc.vector.tensor_copy(qT_sb[:, 2 * it, :],
                                              qps[0:64, it2, :])
                        nc.vector.tensor_copy(qT_sb[:, 2 * it + 1, :],
                                              qps[64:128, it2, :])

                # rope-q: fix token 0 of each chunk (cols ::CS)
                qcols = qT_sb[:, :, :].rearrange(
                    "p h (c w) -> p h c w", w=CS)[:, :, :, 0]   # [64, 8, QG]
                t1q = wk.tile([64, 8, QG], BF16, tag="t1q", bufs=2)
                nc.vector.tensor_mul(
                    t1q, qcols,
                    cosq_sb.unsqueeze(2).broadcast_to((64, 8, QG)))
                t2q = wk.tile([64, 8, QG], BF16, tag="t2q", bufs=2)
                for (dst, src) in ((0, 32), (32, 0)):
                    nc.vector.tensor_mul(
                        t2q[dst:dst + 32, :, :],
                        qT_sb[:, :, :].rearrange(
                            "p h (c w) -> p h c w", w=CS)[src:src + 32, :, :, 0],
                        sinq_sb[src:src + 32, :].unsqueeze(2)
                        .broadcast_to((32, 8, QG)))
                nc.vector.tensor_add(qcols, t1q, t2q)

                # ---- null sims for group: expn_g [1, 8, 512] bf16 ----
                expn_g = wk.tile([1, 8, QG * CS], BF16, tag="expn", bufs=2)
                for h in range(H):
                    nps = pst.tile([1, 4, 128], F32, tag="pst", name=f"nps{g}_{h}")
                    nc.tensor.matmul(
                        nps[:, :, :].rearrange("o a b -> o (a b)"),
                        nullk_bf[:, h:h + 1],
                        qT_sb[:, h, :],
                        start=True, stop=True)
                    nc.scalar.activation(
                        expn_g[:, h, :],
                        nps[:, :, :].rearrange("o a b -> o (a b)"),
                        mybir.ActivationFunctionType.Exp)

                for pp in range(QG // 2):
                    cpair = g * QG + pp * 2   # first chunk of the pair
                    # ---- load ctx pair slices [128, 4, 2, 512] fp8 ----
                    ctx_hi_sb = wk.tile([128, 4, 2, 2 * TK], F8, tag="chi", bufs=2)
                    nc.sync.dma_start(out=ctx_hi_sb, in_=act_re(ctxT_hi)
                                      [:, :, :, cpair * TK:(cpair + 2) * TK])
                    ctx_lo_sb = wk.tile([128, 4, 2, 2 * TK], F8, tag="clo", bufs=2)
                    nc.sync.dma_start(out=ctx_lo_sb, in_=act_re(ctxT_lo)
                                      [:, :, :, cpair * TK:(cpair + 2) * TK])

                    # ---- k projection (fp8 DR 3-term), psum = 32*k ----
                    kraw = wk.tile([64, 8, 2 * TK], BF16, tag="kraw", bufs=2)
                    for ith in range(2):
                        kps = psP.tile([128, 2, 2 * TK], F32, tag="p2",
                                       name=f"kps{cpair}_{ith}")
                        for it2 in range(2):
                            it = ith * 2 + it2
                            mi = 0
                            for (act_t, w_t) in ((ctx_hi_sb, wk_hi_sb),
                                                 (ctx_lo_sb, wk_hi_sb),
                                                 (ctx_hi_sb, wk_lo_sb)):
                                for dt2 in range(4):
                                    nc.tensor.matmul(
                                        kps[:, it2, :],
                                        w_t[:, dt2, :, it * 128:(it + 1) * 128],
                                        act_t[:, dt2, :, :],
                                        start=(mi == 0), stop=(mi == 11),
                                        perf_mode=DR)
                                    mi += 1
                            nc.scalar.copy(kraw[:, 2 * it, :], kps[0:64, it2, :])
                            nc.scalar.copy(kraw[:, 2 * it + 1, :],
                                           kps[64:128, it2, :])

                    # ---- rope-k: DMA rotate-half shift + cos/sin combine ----
                    # tables carry 1/(32*256): kT = rope(k)/256 (q carries 256x)
                    kshift = wk.tile([64, 8, 2 * TK], BF16, tag="kshift", bufs=2)
                    nc.sync.dma_start(out=kshift[0:32, :, :],
                                      in_=kraw[32:64, :, :])
                    nc.sync.dma_start(out=kshift[32:64, :, :],
                                      in_=kraw[0:32, :, :])
                    t1k = wk.tile([64, 8, 2 * TK], BF16, tag="t1k", bufs=1)
                    nc.vector.tensor_mul(
                        t1k[:, :, :].rearrange("p h (rep c) -> p h rep c", rep=4),
                        kraw[:, :, :].rearrange("p h (rep c) -> p h rep c", rep=4),
                        cosk_sb.unsqueeze(1).unsqueeze(2)
                        .broadcast_to((64, 8, 4, 128)))
                    t2k = wk.tile([64, 8, 2 * TK], BF16, tag="t2k", bufs=1)
                    nc.vector.tensor_mul(
                        t2k[:, :, :].rearrange("p h (rep c) -> p h rep c", rep=4),
                        kshift[:, :, :].rearrange("p h (rep c) -> p h rep c", rep=4),
                        sink_sb.unsqueeze(1).unsqueeze(2)
                        .broadcast_to((64, 8, 4, 128)))
                    kT_bf = wk.tile([64, 8, 2 * TK], BF16, tag="kT", bufs=2)
                    nc.vector.tensor_add(kT_bf, t1k, t2k)

                    # two chunks of attention per pair
                    for sub in range(2):
                        cc = pp * 2 + sub
                        c = g * QG + cc
                        # ---- v projection (fp8 DR 3-term): v_aug = psum/32 ----
                        vps = psP.tile([128, 2, INNER], F32, tag="p2",
                                       name=f"vps{c}")
                        for tg in range(2):
                            mi = 0
                            for (act_t, w_t) in ((ctx_hi_sb, wv_hi_sb),
                                                 (ctx_lo_sb, wv_hi_sb),
                                                 (ctx_hi_sb, wv_lo_sb)):
                                for dt2 in range(4):
                                    nc.tensor.matmul(
                                        vps[:, tg, :],
                                        act_t[:, dt2, :,
                                              sub * TK + tg * 128:
                                              sub * TK + (tg + 1) * 128],
                                        w_t[:, dt2, :, :],
                                        start=(mi == 0), stop=(mi == 11),
                                        perf_mode=DR)
                                    mi += 1
                        v_aug = wk.tile([128, 2, 8, 65], BF16, tag="v_aug", bufs=2)
                        nc.scalar.activation(
                            v_aug[:, :, :, 0:64],
                            vps[:, :, :].rearrange("p tg (h w) -> p tg h w", h=8),
                            mybir.ActivationFunctionType.Copy,
                            scale=1.0 / WS_KV)
                        nc.gpsimd.memset(v_aug[:, :, :, 64:65], 1.0)

                        # ---- sim matmuls: simT [128j, 2jg, (h,i)] ----
                        sps = psP.tile([128, 2, 512], F32, tag="p2", name=f"sps{c}")
                        for h in range(H):
                            for jg in range(2):
                                nc.tensor.matmul(
                                    sps[:, jg, h * 64:(h + 1) * 64],
                                    kT_bf[:, h, sub * TK + jg * 128:
                                          sub * TK + (jg + 1) * 128],
                                    qT_sb[:, h, cc * CS:(cc + 1) * CS],
                                    start=True, stop=True)
                        expT = wk.tile([128, 2, 512], BF16, tag="expT", bufs=2)
                        nc.scalar.activation(expT, sps,
                                             mybir.ActivationFunctionType.Exp)

                        # ---- o matmuls [64i, 65] per head (col 64 = sum) ----
                        ops_ = psP.tile([64, 8, 65], F32, tag="p2", name=f"ops{c}")
                        for h in range(H):
                            dst = ops_[:, h, :]
                            for jg in range(2):
                                nc.tensor.matmul(
                                    dst,
                                    expT[:, jg, h * 64:(h + 1) * 64],
                                    v_aug[:, jg, h, :],
                                    start=(jg == 0), stop=False)
                            nc.tensor.matmul(
                                dst,
                                expn_g[0:1, h, cc * CS:(cc + 1) * CS],
                                nullv_bf[0:1, h, :],
                                start=False, stop=True)

                        # ---- normalize (batched) into pair buffer ----
                        rcol = wk.tile([64, 8], F32, tag="rcol", bufs=2)
                        nc.vector.reciprocal(rcol, ops_[:, :, 64])
                        if sub == 0:
                            o_pair = wk.tile([128, 8, 64], BF16, tag="o_pair",
                                             bufs=2)
                        nc.vector.tensor_mul(
                            o_pair[sub * 64:(sub + 1) * 64, :, :],
                            ops_[:, :, 0:64],
                            rcol.unsqueeze(2).broadcast_to((64, 8, 64)))

                        if sub == 0:
                            continue
                        # ---- transpose o pair -> oT [128e, 4et, 128t] ----
                        otr = pst.tile([128, 4, 256], BF16, tag="pst",
                                       name=f"otr{cpair}")
                        for et in range(4):
                            nc.tensor.transpose(
                                otr[:, et, 0:128],
                                o_pair[:, 2 * et:2 * et + 2, :],
                                ident)
                        oT_sb = wk.tile([128, 4, 128], F32, tag="oT", bufs=2)
                        nc.vector.tensor_copy(oT_sb, otr[:, :, 0:128])

                        # ---- out projection + bias (pair, M=128) ----
                        outps = psP.tile([128, DIM], F32, tag="p2",
                                         name=f"outps{cpair}")
                        for co in range(2):
                            for et in range(4):
                                nc.tensor.matmul(
                                    outps[:, co * 512:(co + 1) * 512],
                                    oT_sb[:, et, :].bitcast(F32R),
                                    wo_sb[:, et, co * 512:(co + 1) * 512],
                                    start=(et == 0), stop=(et == 3))
                        out_sb = wk.tile([128, DIM], BF16, tag="out_sb", bufs=2)
                        nc.gpsimd.tensor_add(out_sb, outps, bo_sb)
                        nc.sync.dma_start(
                            out=out[cpair * CS:(cpair + 2) * CS, :], in_=out_sb)

    nc.compile()
    return nc


_CACHED_NC = None


def _get_nc():
    global _CACHED_NC
    if _CACHED_NC is None:
        _CACHED_NC = _build_bass()
    return _CACHED_NC


def _split8(a):
    hi = np.asarray(a, np.float32).astype(ml_dtypes.float8_e4m3fn)
    lo = (np.asarray(a, np.float32) - hi.astype(np.float32)).astype(
        ml_dtypes.float8_e4m3fn)
    return hi, lo


def kernel(x, context, q_pos_emb, k_pos_emb, Wq, Wk, Wv, Wo, bo, null_k, null_v):
    x = np.asarray(x, dtype=np.float32)
    context = np.asarray(context, dtype=np.float32)
    q_pos_emb = np.asarray(q_pos_emb, dtype=np.float32)
    k_pos_emb = np.asarray(k_pos_emb, dtype=np.float32)
    Wq = np.asarray(Wq, dtype=np.float32)
    Wk = np.asarray(Wk, dtype=np.float32)
    Wv = np.asarray(Wv, dtype=np.float32)
    Wo = np.asarray(Wo, dtype=np.float32)
    bo = np.asarray(bo, dtype=np.float32)
    null_k = np.asarray(null_k, dtype=np.float32)
    null_v = np.asarray(null_v, dtype=np.float32)

    # ---- host marshalling (layout/dtype only + tiny rope tables) ----
    xs = np.zeros_like(x)
    xs[:, : N - CP] = x[:, CP:]
    xc = xs.reshape(BK, CS, DIM)
    ctx = context.reshape(BK, TK, DIM)

    wq_hi, wq_lo = _split8(Wq * (SCALE * WS_Q))
    wk_hi, wk_lo = _split8(Wk * WS_KV)
    wv_hi, wv_lo = _split8(Wv * WS_KV)

    qpe63 = q_pos_emb[0, 0, CP]
    cos_q0 = np.cos(qpe63)[:, None].astype(np.float32)          # [64, 1]
    sgn = np.where(np.arange(64) < 32, -1.0, 1.0)
    sin_q0s = (np.sin(qpe63) * sgn)[:, None].astype(np.float32)
    # permuted so the partition-shifted mul reads table at the src base
    # partition (BIR requires equal base partitions for two SBUF inputs)
    sp = np.empty_like(sin_q0s)
    sp[0:32] = sin_q0s[32:64]; sp[32:64] = sin_q0s[0:32]
    sin_q0s = sp

    kpe = k_pos_emb[0, 0]
    ks = 1.0 / (WS_KV * WS_Q)        # fold psum x32 and q x256 into tables
    cos_kT = np.ascontiguousarray((np.cos(kpe.T) * ks).astype(np.float32))
    # sign of rotate-half lands on dst rows 0-31 (out[d<32] = -k[d+32]*sin)
    sgn_k = np.where(np.arange(64) < 32, -1.0, 1.0)[:, None]
    sin_kT = np.ascontiguousarray((np.sin(kpe.T) * ks * sgn_k).astype(np.float32))

    nullkT = np.ascontiguousarray(
        (null_k.reshape(8, 64).T / WS_Q).astype(np.float32))       # [64, 8]
    nullv_a = np.zeros((1, 8, 65), np.float32)
    nullv_a[0, :, :64] = null_v.reshape(8, 64)
    nullv_a[0, :, 64] = 1.0
    nullv_a = nullv_a.reshape(1, 8 * 65)

    shared = {
        "wq_hi": wq_hi, "wq_lo": wq_lo, "wk_hi": wk_hi, "wk_lo": wk_lo,
        "wv_hi": wv_hi, "wv_lo": wv_lo, "Wo": Wo, "bo": bo,
        "cos_kT": cos_kT, "sin_kT": sin_kT,
        "nullkT": nullkT, "nullv_aug": nullv_a,
        "cos_q0": cos_q0, "sin_q0s": sin_q0s,
    }
    in_maps = []
    for c in range(N_CORES):
        sl = slice(c * CPC, (c + 1) * CPC)
        xT_c = np.ascontiguousarray(xc[sl].reshape(TQ, DIM).T)
        ctxT_c = np.ascontiguousarray(ctx[sl].reshape(TCTX, DIM).T)
        x_hi, x_lo = _split8(xT_c)
        c_hi, c_lo = _split8(ctxT_c)
        in_maps.append({"xT_hi": x_hi, "xT_lo": x_lo,
                        "ctxT_hi": c_hi, "ctxT_lo": c_lo, **shared})

    nc = _get_nc()
    res = run_bass_kernel_spmd(nc, in_maps, core_ids=list(range(N_CORES)))

    out_full = np.concatenate(
        [np.asarray(res.results[c]["out"], np.float32) for c in range(N_CORES)],
        axis=0)                                           # [BK*CS, DIM]
    o = out_full.reshape(B, K_CHUNKS * CS, DIM)
    final = np.concatenate(
        [np.zeros((B, CP, DIM), np.float32), o[:, : K_CHUNKS * CS - CP]], axis=1)
    return final


# revision 4
# speedup vs baseline: 1.6470x; 1.4593x over previous
"""Trainium2 Bass kernel for nn_ChunkedCrossAttention_85907935855128.

Self-contained: hardcodes shapes/sharding. Accepts FULL inputs, returns FULL output.
Shards the fused (b*k_chunks) chunk axis across 8 NeuronCores; weights replicated.

Per-core dataflow:
  q/k/v projections run as fp8-E4M3 DoubleRow matmuls with a 3-term hi/lo
  error-compensation split (act_hi@W_hi + act_lo@W_hi + act_hi@W_lo), which
  the host prepares: activations split into e4m3 hi + residual lo, weights
  pre-scaled (x32 for Wk/Wv, x256*SCALE for Wq) so they quantize outside the
  e4m3 denormal range. Scale compensation is folded into constants: 1/(32*256)
  into the k rope tables, 1/256 into null_k, 1/32 into the v psum->sbuf copy.
  Rope on k = cos*k + sin*shift(k) where the rotate-half shift is done by an
  SBUF->SBUF DMA partition swap (keeps it off the PE/DVE critical path).
  Attention (sim/exp/o) in bf16 with softmax sums via a ones column in v_aug;
  out-projection in f32r; bias add + bf16 cast on gpsimd; bf16 output.
  The attention stage is software-pipelined one chunk-pair behind the
  projections so the PE never waits on the Act/DMA/DVE rope chain.
"""
import os
# bass2jax executes via the axon PJRT platform; a CPU pin would hide the cores.
if os.environ.get("JAX_PLATFORMS", "") in ("cpu",):
    del os.environ["JAX_PLATFORMS"]

import numpy as np
import ml_dtypes

import concourse.bacc as bacc
import concourse.bass as bass
import concourse.mybir as mybir
import concourse.tile as tile
from concourse.bass_utils import run_bass_kernel_spmd
from concourse.masks import make_identity

F32 = mybir.dt.float32
F32R = mybir.dt.float32r
BF16 = mybir.dt.bfloat16
F8 = mybir.dt.float8e4
DR = mybir.MatmulPerfMode.DoubleRow
EXP = mybir.ActivationFunctionType.Exp
COPY = mybir.ActivationFunctionType.Copy

CS, CP, H, DH = 64, 63, 8, 64
SCALE = DH ** -0.5
N_CORES = 8
B, N, DIM = 4, 4096, 1024
K_CHUNKS, R, RLEN = 64, 2, 128
TK = R * RLEN                 # 256 ctx tokens / chunk
BK = B * K_CHUNKS             # 256 chunks
CPC = BK // N_CORES           # 32 chunks / core
TQ = CPC * CS                 # 2048 q tokens / core
TCTX = CPC * TK               # 8192 ctx tokens / core
INNER = H * DH                # 512
QG = 8                        # chunks per q-projection group (512 tokens)
NQG = CPC // QG               # 4 q groups / core
NPAIR = CPC // 2              # 16 chunk-pairs / core
WS_KV = 32.0                  # host pre-scale on Wk/Wv before e4m3
WS_Q = 256.0                  # host pre-scale on Wq*SCALE before e4m3


def _build_bass(num_devices=N_CORES):
    nc = bacc.Bacc("TRN2", target_bir_lowering=False, debug=False,
                   num_devices=num_devices)

    xT_hi = nc.dram_tensor("xT_hi", (DIM, TQ), F8, kind="ExternalInput")
    xT_lo = nc.dram_tensor("xT_lo", (DIM, TQ), F8, kind="ExternalInput")
    ctxT_hi = nc.dram_tensor("ctxT_hi", (DIM, TCTX), F8, kind="ExternalInput")
    ctxT_lo = nc.dram_tensor("ctxT_lo", (DIM, TCTX), F8, kind="ExternalInput")
    wq_hi = nc.dram_tensor("wq_hi", (DIM, INNER), F8, kind="ExternalInput")
    wq_lo = nc.dram_tensor("wq_lo", (DIM, INNER), F8, kind="ExternalInput")
    wk_hi = nc.dram_tensor("wk_hi", (DIM, INNER), F8, kind="ExternalInput")
    wk_lo = nc.dram_tensor("wk_lo", (DIM, INNER), F8, kind="ExternalInput")
    wv_hi = nc.dram_tensor("wv_hi", (DIM, INNER), F8, kind="ExternalInput")
    wv_lo = nc.dram_tensor("wv_lo", (DIM, INNER), F8, kind="ExternalInput")
    Wo = nc.dram_tensor("Wo", (INNER, DIM), F32, kind="ExternalInput")
    bo = nc.dram_tensor("bo", (DIM,), F32, kind="ExternalInput")
    cos_kT = nc.dram_tensor("cos_kT", (64, 128), F32, kind="ExternalInput")
    sin_kT = nc.dram_tensor("sin_kT", (64, 128), F32, kind="ExternalInput")
    nullkT = nc.dram_tensor("nullkT", (64, 8), F32, kind="ExternalInput")
    nullv_aug = nc.dram_tensor("nullv_aug", (1, 8 * 65), F32, kind="ExternalInput")
    cos_q0 = nc.dram_tensor("cos_q0", (64, 1), F32, kind="ExternalInput")
    sin_q0s = nc.dram_tensor("sin_q0s", (64, 1), F32, kind="ExternalInput")
    out = nc.dram_tensor("out", (TQ, DIM), BF16, kind="ExternalOutput")

    def w_re(t):
        return t[:, :].rearrange("(dt2 kt p) i -> p dt2 kt i", p=128, kt=2)

    def act_re(t):
        return t[:, :].rearrange("(dt2 kt p) t -> p dt2 kt t", p=128, kt=2)

    with tile.TileContext(nc) as tc:
        with tc.tile_pool(name="consts", bufs=1) as cp_, \
             tc.tile_pool(name="wk", bufs=2) as wk, \
             tc.tile_pool(name="psP", bufs=3, space="PSUM") as psP, \
             tc.tile_pool(name="pst", bufs=2, space="PSUM") as pst:

            # ---- constants ----
            wq_hi_sb = cp_.tile([128, 4, 2, INNER], F8)
            nc.sync.dma_start(out=wq_hi_sb, in_=w_re(wq_hi))
            wq_lo_sb = cp_.tile([128, 4, 2, INNER], F8)
            nc.sync.dma_start(out=wq_lo_sb, in_=w_re(wq_lo))
            wk_hi_sb = cp_.tile([128, 4, 2, INNER], F8)
            nc.sync.dma_start(out=wk_hi_sb, in_=w_re(wk_hi))
            wk_lo_sb = cp_.tile([128, 4, 2, INNER], F8)
            nc.sync.dma_start(out=wk_lo_sb, in_=w_re(wk_lo))
            wv_hi_sb = cp_.tile([128, 4, 2, INNER], F8)
            nc.sync.dma_start(out=wv_hi_sb, in_=w_re(wv_hi))
            wv_lo_sb = cp_.tile([128, 4, 2, INNER], F8)
            nc.sync.dma_start(out=wv_lo_sb, in_=w_re(wv_lo))
            wo_sb = cp_.tile([128, 4, DIM], F32R)
            nc.sync.dma_start(out=wo_sb, in_=Wo[:, :].rearrange(
                "(et p) c -> p et c", p=128).bitcast(F32R))

            bo_sb = cp_.tile([128, DIM], F32)
            nc.sync.dma_start(out=bo_sb, in_=bass.AP(
                tensor=bo, offset=0, ap=[[0, 128], [1, DIM]]))

            cosk_f = cp_.tile([64, 128], F32)
            nc.sync.dma_start(out=cosk_f, in_=cos_kT[:, :])
            cosk_sb = cp_.tile([64, 128], BF16)
            nc.vector.tensor_copy(cosk_sb, cosk_f)
            sink_f = cp_.tile([64, 128], F32)
            nc.sync.dma_start(out=sink_f, in_=sin_kT[:, :])
            sink_sb = cp_.tile([64, 128], BF16)
            nc.vector.tensor_copy(sink_sb, sink_f)

            cosq_sb = cp_.tile([64, 1], F32)
            nc.sync.dma_start(out=cosq_sb, in_=cos_q0[:, :])
            sinq_sb = cp_.tile([64, 1], F32)
            nc.sync.dma_start(out=sinq_sb, in_=sin_q0s[:, :])

            nullk_f32 = cp_.tile([64, 8], F32)
            nc.sync.dma_start(out=nullk_f32, in_=nullkT[:, :])
            nullk_bf = cp_.tile([64, 8], BF16)
            nc.vector.tensor_copy(nullk_bf, nullk_f32)

            nullv_f32 = cp_.tile([1, 8, 65], F32)
            nc.sync.dma_start(out=nullv_f32, in_=nullv_aug[:, :].rearrange(
                "o (h w) -> o h w", h=8))
            nullv_bf = cp_.tile([1, 8, 65], BF16)
            nc.vector.tensor_copy(nullv_bf, nullv_f32)

            ident = cp_.tile([128, 128], BF16)
            make_identity(nc, ident)

            def load_ctx(cpair):
                hi = wk.tile([128, 4, 2, 2 * TK], F8, tag="chi", bufs=2)
                nc.sync.dma_start(out=hi, in_=act_re(ctxT_hi)
                                  [:, :, :, cpair * TK:(cpair + 2) * TK])
                lo = wk.tile([128, 4, 2, 2 * TK], F8, tag="clo", bufs=2)
                nc.sync.dma_start(out=lo, in_=act_re(ctxT_lo)
                                  [:, :, :, cpair * TK:(cpair + 2) * TK])
                return hi, lo

            def q_stage(g):
                x_hi_sb = wk.tile([128, 4, 2, QG * CS], F8, tag="xhi", bufs=2)
                nc.sync.dma_start(out=x_hi_sb, in_=act_re(xT_hi)
                                  [:, :, :, g * QG * CS:(g + 1) * QG * CS])
                x_lo_sb = wk.tile([128, 4, 2, QG * CS], F8, tag="xlo", bufs=2)
                nc.sync.dma_start(out=x_lo_sb, in_=act_re(xT_lo)
                                  [:, :, :, g * QG * CS:(g + 1) * QG * CS])
                qT_sb = wk.tile([64, 8, QG * CS], BF16, tag="qT", bufs=2)
                for ith in range(2):
                    qps = psP.tile([128, 2, QG * CS], F32, tag="p2",
                                   name=f"qps{g}_{ith}")
                    for it2 in range(2):
                        it = ith * 2 + it2
                        mi = 0
                        for (act_t, w_t) in ((x_hi_sb, wq_hi_sb),
                                             (x_lo_sb, wq_hi_sb),
                                             (x_hi_sb, wq_lo_sb)):
                            for dt2 in range(4):
                                nc.tensor.matmul(
                                    qps[:, it2, :],
                                    w_t[:, dt2, :, it * 128:(it + 1) * 128],
                                    act_t[:, dt2, :, :],
                                    start=(mi == 0), stop=(mi == 11),
                                    perf_mode=DR)
                                mi += 1
                        nc.vector.tensor_copy(qT_sb[:, 2 * it, :],
                                              qps[0:64, it2, :])
                        nc.vector.tensor_copy(qT_sb[:, 2 * it + 1, :],
                                              qps[64:128, it2, :])

                # rope-q: fix token 0 of each chunk (cols ::CS)
                qcols = qT_sb[:, :, :].rearrange(
                    "p h (c w) -> p h c w", w=CS)[:, :, :, 0]   # [64, 8, QG]
                t1q = wk.tile([64, 8, QG], BF16, tag="t1q", bufs=2)
                nc.vector.tensor_mul(
                    t1q, qcols,
                    cosq_sb.unsqueeze(2).broadcast_to((64, 8, QG)))
                t2q = wk.tile([64, 8, QG], BF16, tag="t2q", bufs=2)
                for (dst, src) in ((0, 32), (32, 0)):
                    nc.vector.tensor_mul(
                        t2q[dst:dst + 32, :, :],
                        qT_sb[:, :, :].rearrange(
                            "p h (c w) -> p h c w", w=CS)[src:src + 32, :, :, 0],
                        sinq_sb[src:src + 32, :].unsqueeze(2)
                        .broadcast_to((32, 8, QG)))
                nc.vector.tensor_add(qcols, t1q, t2q)

                # null sims for group: expn_g [1, 8, 512] bf16
                expn_g = wk.tile([1, 8, QG * CS], BF16, tag="expn", bufs=2)
                for h in range(H):
                    nps = pst.tile([1, 4, 128], F32, tag="pst", name=f"nps{g}_{h}")
                    nc.tensor.matmul(
                        nps[:, :, :].rearrange("o a b -> o (a b)"),
                        nullk_bf[:, h:h + 1],
                        qT_sb[:, h, :],
                        start=True, stop=True)
                    nc.scalar.activation(
                        expn_g[:, h, :],
                        nps[:, :, :].rearrange("o a b -> o (a b)"),
                        EXP)
                return qT_sb, expn_g

            def k_proj(cpair, ctx_hi_sb, ctx_lo_sb):
                """fp8 DR 3-term k projection + rope via DMA shift; psum = 32*k,
                tables carry 1/(32*256): kT = rope(k)/256 (q carries 256x)."""
                kraw = wk.tile([64, 8, 2 * TK], BF16, tag="kraw", bufs=2)
                for ith in range(2):
                    kps = psP.tile([128, 2, 2 * TK], F32, tag="p2",
                                   name=f"kps{cpair}_{ith}")
                    for it2 in range(2):
                        it = ith * 2 + it2
                        mi = 0
                        for (act_t, w_t) in ((ctx_hi_sb, wk_hi_sb),
                                             (ctx_lo_sb, wk_hi_sb),
                                             (ctx_hi_sb, wk_lo_sb)):
                            for dt2 in range(4):
                                nc.tensor.matmul(
                                    kps[:, it2, :],
                                    w_t[:, dt2, :, it * 128:(it + 1) * 128],
                                    act_t[:, dt2, :, :],
                                    start=(mi == 0), stop=(mi == 11),
                                    perf_mode=DR)
                                mi += 1
                        nc.scalar.copy(kraw[:, 2 * it, :], kps[0:64, it2, :])
                        nc.scalar.copy(kraw[:, 2 * it + 1, :],
                                       kps[64:128, it2, :])

                kshift = wk.tile([64, 8, 2 * TK], BF16, tag="kshift", bufs=2)
                nc.sync.dma_start(out=kshift[0:32, :, :], in_=kraw[32:64, :, :])
                nc.sync.dma_start(out=kshift[32:64, :, :], in_=kraw[0:32, :, :])
                t1k = wk.tile([64, 8, 2 * TK], BF16, tag="t1k", bufs=1)
                nc.vector.tensor_mul(
                    t1k[:, :, :].rearrange("p h (rep c) -> p h rep c", rep=4),
                    kraw[:, :, :].rearrange("p h (rep c) -> p h rep c", rep=4),
                    cosk_sb.unsqueeze(1).unsqueeze(2)
                    .broadcast_to((64, 8, 4, 128)))
                t2k = wk.tile([64, 8, 2 * TK], BF16, tag="t2k", bufs=1)
                nc.vector.tensor_mul(
                    t2k[:, :, :].rearrange("p h (rep c) -> p h rep c", rep=4),
                    kshift[:, :, :].rearrange("p h (rep c) -> p h rep c", rep=4),
                    sink_sb.unsqueeze(1).unsqueeze(2)
                    .broadcast_to((64, 8, 4, 128)))
                kT_bf = wk.tile([64, 8, 2 * TK], BF16, tag="kT", bufs=2)
                nc.vector.tensor_add(kT_bf, t1k, t2k)
                return kT_bf

            def v_proj(c, sub, ctx_hi_sb, ctx_lo_sb):
                """fp8 DR 3-term v projection for one chunk: v_aug = psum/32."""
                vps = psP.tile([128, 2, INNER], F32, tag="p2", name=f"vps{c}")
                for tg in range(2):
                    mi = 0
                    for (act_t, w_t) in ((ctx_hi_sb, wv_hi_sb),
                                         (ctx_lo_sb, wv_hi_sb),
                                         (ctx_hi_sb, wv_lo_sb)):
                        for dt2 in range(4):
                            nc.tensor.matmul(
                                vps[:, tg, :],
                                act_t[:, dt2, :,
                                      sub * TK + tg * 128:
                                      sub * TK + (tg + 1) * 128],
                                w_t[:, dt2, :, :],
                                start=(mi == 0), stop=(mi == 11),
                                perf_mode=DR)
                            mi += 1
                v_aug = wk.tile([128, 2, 8, 65], BF16, tag="v_aug", bufs=4)
                nc.scalar.activation(
                    v_aug[:, :, :, 0:64],
                    vps[:, :, :].rearrange("p tg (h w) -> p tg h w", h=8),
                    COPY, scale=1.0 / WS_KV)
                nc.gpsimd.memset(v_aug[:, :, :, 64:65], 1.0)
                return v_aug

            def attn_sim(P):
                """sim + exp for the pending pair."""
                P["expT"] = []
                for sub in range(2):
                    cc = (P["cpair"] % QG) + sub
                    sps = psP.tile([128, 2, 512], F32, tag="p2",
                                   name=f"sps{P['cpair']}_{sub}")
                    for h in range(H):
                        for jg in range(2):
                            nc.tensor.matmul(
                                sps[:, jg, h * 64:(h + 1) * 64],
                                P["kT"][:, h, sub * TK + jg * 128:
                                        sub * TK + (jg + 1) * 128],
                                P["qT"][:, h, cc * CS:(cc + 1) * CS],
                                start=True, stop=True)
                    expT = wk.tile([128, 2, 512], BF16, tag="expT", bufs=2)
                    nc.scalar.activation(expT, sps, EXP)
                    P["expT"].append(expT)

            def attn_out(P):
                """o matmuls, normalize, transpose, out projection, store."""
                cpair = P["cpair"]
                o_pair = wk.tile([128, 8, 64], BF16, tag="o_pair", bufs=2)
                for sub in range(2):
                    cc = (cpair % QG) + sub
                    expT = P["expT"][sub]
                    v_aug = P["v_aug"][sub]
                    ops_ = psP.tile([64, 8, 65], F32, tag="p2",
                                    name=f"ops{cpair}_{sub}")
                    for h in range(H):
                        dst = ops_[:, h, :]
                        for jg in range(2):
                            nc.tensor.matmul(
                                dst,
                                expT[:, jg, h * 64:(h + 1) * 64],
                                v_aug[:, jg, h, :],
                                start=(jg == 0), stop=False)
                        nc.tensor.matmul(
                            dst,
                            P["expn"][0:1, h, cc * CS:(cc + 1) * CS],
                            nullv_bf[0:1, h, :],
                            start=False, stop=True)
                    rcol = wk.tile([64, 8], F32, tag="rcol", bufs=2)
                    nc.vector.reciprocal(rcol, ops_[:, :, 64])
                    nc.vector.tensor_mul(
                        o_pair[sub * 64:(sub + 1) * 64, :, :],
                        ops_[:, :, 0:64],
                        rcol.unsqueeze(2).broadcast_to((64, 8, 64)))

                otr = pst.tile([128, 4, 256], BF16, tag="pst", name=f"otr{cpair}")
                for et in range(4):
                    nc.tensor.transpose(
                        otr[:, et, 0:128],
                        o_pair[:, 2 * et:2 * et + 2, :],
                        ident)
                oT_sb = wk.tile([128, 4, 128], F32, tag="oT", bufs=2)
                nc.vector.tensor_copy(oT_sb, otr[:, :, 0:128])

                outps = psP.tile([128, DIM], F32, tag="p2", name=f"outps{cpair}")
                for co in range(2):
                    for et in range(4):
                        nc.tensor.matmul(
                            outps[:, co * 512:(co + 1) * 512],
                            oT_sb[:, et, :].bitcast(F32R),
                            wo_sb[:, et, co * 512:(co + 1) * 512],
                            start=(et == 0), stop=(et == 3))
                out_sb = wk.tile([128, DIM], BF16, tag="out_sb", bufs=2)
                nc.gpsimd.tensor_add(out_sb, outps, bo_sb)
                nc.sync.dma_start(
                    out=out[cpair * CS:(cpair + 2) * CS, :], in_=out_sb)

            # ---- software-pipelined main loop: attention runs 1 pair behind
            pend = None
            ctx_cur = load_ctx(0)
            for g in range(NQG):
                qT_sb, expn_g = q_stage(g)
                for pp in range(QG // 2):
                    cpair = g * QG + pp * 2
                    pidx = g * (QG // 2) + pp
                    ctx_next = load_ctx(cpair + 2) if pidx < NPAIR - 1 else None
                    kT_bf = k_proj(cpair, *ctx_cur)
                    if pend is not None:
                        attn_sim(pend)
                    va = [v_proj(cpair + sub, sub, *ctx_cur) for sub in range(2)]
                    if pend is not None:
                        attn_out(pend)
                    pend = {"cpair": cpair, "kT": kT_bf, "v_aug": va,
                            "qT": qT_sb, "expn": expn_g}
                    ctx_cur = ctx_next
            attn_sim(pend)
            attn_out(pend)

    nc.compile()
    return nc


_CACHED_NC = None


def _get_nc():
    global _CACHED_NC
    if _CACHED_NC is None:
        _CACHED_NC = _build_bass()
    return _CACHED_NC


def _split8(a):
    hi = np.asarray(a, np.float32).astype(ml_dtypes.float8_e4m3fn)
    lo = (np.asarray(a, np.float32) - hi.astype(np.float32)).astype(
        ml_dtypes.float8_e4m3fn)
    return hi, lo


def kernel(x, context, q_pos_emb, k_pos_emb, Wq, Wk, Wv, Wo, bo, null_k, null_v):
    x = np.asarray(x, dtype=np.float32)
    context = np.asarray(context, dtype=np.float32)
    q_pos_emb = np.asarray(q_pos_emb, dtype=np.float32)
    k_pos_emb = np.asarray(k_pos_emb, dtype=np.float32)
    Wq = np.asarray(Wq, dtype=np.float32)
    Wk = np.asarray(Wk, dtype=np.float32)
    Wv = np.asarray(Wv, dtype=np.float32)
    Wo = np.asarray(Wo, dtype=np.float32)
    bo = np.asarray(bo, dtype=np.float32)
    null_k = np.asarray(null_k, dtype=np.float32)
    null_v = np.asarray(null_v, dtype=np.float32)

    # ---- host marshalling (layout/dtype only + tiny rope tables) ----
    xs = np.zeros_like(x)
    xs[:, : N - CP] = x[:, CP:]
    xc = xs.reshape(BK, CS, DIM)
    ctx = context.reshape(BK, TK, DIM)

    wq_hi, wq_lo = _split8(Wq * (SCALE * WS_Q))
    wk_hi, wk_lo = _split8(Wk * WS_KV)
    wv_hi, wv_lo = _split8(Wv * WS_KV)

    qpe63 = q_pos_emb[0, 0, CP]
    cos_q0 = np.cos(qpe63)[:, None].astype(np.float32)          # [64, 1]
    sgn = np.where(np.arange(64) < 32, -1.0, 1.0)
    sin_q0s = (np.sin(qpe63) * sgn)[:, None].astype(np.float32)
    # permuted so the partition-shifted mul reads table at the src base
    # partition (BIR requires equal base partitions for two SBUF inputs)
    sp = np.empty_like(sin_q0s)
    sp[0:32] = sin_q0s[32:64]; sp[32:64] = sin_q0s[0:32]
    sin_q0s = sp

    kpe = k_pos_emb[0, 0]
    ks = 1.0 / (WS_KV * WS_Q)        # fold psum x32 and q x256 into tables
    cos_kT = np.ascontiguousarray((np.cos(kpe.T) * ks).astype(np.float32))
    # sign of rotate-half lands on dst rows 0-31 (out[d<32] = -k[d+32]*sin)
    sgn_k = np.where(np.arange(64) < 32, -1.0, 1.0)[:, None]
    sin_kT = np.ascontiguousarray((np.sin(kpe.T) * ks * sgn_k).astype(np.float32))

    nullkT = np.ascontiguousarray(
        (null_k.reshape(8, 64).T / WS_Q).astype(np.float32))       # [64, 8]
    nullv_a = np.zeros((1, 8, 65), np.float32)
    nullv_a[0, :, :64] = null_v.reshape(8, 64)
    nullv_a[0, :, 64] = 1.0
    nullv_a = nullv_a.reshape(1, 8 * 65)

    shared = {
        "wq_hi": wq_hi, "wq_lo": wq_lo, "wk_hi": wk_hi, "wk_lo": wk_lo,
        "wv_hi": wv_hi, "wv_lo": wv_lo, "Wo": Wo, "bo": bo,
        "cos_kT": cos_kT, "sin_kT": sin_kT,
        "nullkT": nullkT, "nullv_aug": nullv_a,
        "cos_q0": cos_q0, "sin_q0s": sin_q0s,
    }
    in_maps = []
    for c in range(N_CORES):
        sl = slice(c * CPC, (c + 1) * CPC)
        xT_c = np.ascontiguousarray(xc[sl].reshape(TQ, DIM).T)
        ctxT_c = np.ascontiguousarray(ctx[sl].reshape(TCTX, DIM).T)
        x_hi, x_lo = _split8(xT_c)
        c_hi, c_lo = _split8(ctxT_c)
        in_maps.append({"xT_hi": x_hi, "xT_lo": x_lo,
                        "ctxT_hi": c_hi, "ctxT_lo": c_lo, **shared})

    nc = _get_nc()
    res = run_bass_kernel_spmd(nc, in_maps, core_ids=list(range(N_CORES)))

    out_full = np.concatenate(
        [np.asarray(res.results[c]["out"], np.float32) for c in range(N_CORES)],
        axis=0)                                           # [BK*CS, DIM]
    o = out_full.reshape(B, K_CHUNKS * CS, DIM)
    final = np.concatenate(
        [np.zeros((B, CP, DIM), np.float32), o[:, : K_CHUNKS * CS - CP]], axis=1)
    return final
